# revision 31
# baseline (speedup 1.0000x reference)
"""Trainium2 Bass kernel for nn_DecoderTrans (dense transformer decoder layer + vocab head).

Sharding: 8 cores = (batch b, half hf). Each core computes the full trunk for its
512 "own" tokens (queries) and the K/V context for the whole 1024-token sequence
of its batch element. Own tokens always occupy key slots [512, 1024) so the
program is uniform SPMD; per-core mask/bias DATA encodes the causal structure.
Activations are kept feature-major (x^T: [D, tokens]) throughout; weights are
shipped pre-transposed ([d_in, d_out]) in bf16 (PSUM accumulation stays fp32).
"""
import math
import os
import sys

sys.path.insert(0, "/opt/trn_rl_repo")

import numpy as np

import concourse.bass as bass
import concourse.tile as tile
from concourse import bacc, mybir
from concourse.bass import ts
from concourse.masks import make_identity

P = 128
D = 512
DC = D // P          # 4 feature chunks
T = 1024             # full sequence (keys)
TOWN = 512           # own tokens per core (queries), slots [512, 1024)
H = 8
DKH = 64             # head dim
V = 32000
VCH = 500            # vocab columns per matmul (fits PSUM bank)
VG = 4               # vocab chunks per group
NVG = V // (VCH * VG)  # 16 groups
FFN = 2 * D
NEG = -30000.0
SQRT_D = math.sqrt(D)
PAD_ID = 0

F32 = mybir.dt.float32
I32 = mybir.dt.int32
AF = mybir.ActivationFunctionType
OP = mybir.AluOpType

BF16 = mybir.dt.bfloat16
MF = BF16            # matmul operand dtype (activations + weights)
OUT_DT = BF16        # output staging dtype (halves the 32MB/core logit writeback)

# packed fp32 constant block column offsets: biasS(8) biasC(8) then
# per-projection bias columns and layernorm gain/bias packs
_CB = {}
_off = 0
for _nm, _n in (("vmS", 8), ("vmC", 8), ("bq", DC), ("bk", DC),
                ("bo1", DC), ("cbq", DC), ("ebk", DC), ("bo2", DC),
                ("b2", DC), ("b1", FFN // P),
                ("gc1", DC), ("bc1", DC), ("gc2", DC), ("bc2", DC),
                ("gc3", DC), ("bc3", DC)):
    _CB[_nm] = (_off, _n)
    _off += _n
NCONST = _off


# --------------------------------------------------------------------------
# program builder
# --------------------------------------------------------------------------

def build_module():
    nc = bacc.Bacc("TRN2", target_bir_lowering=False, debug=False)

    def din(name, shape, dt=F32):
        return nc.dram_tensor(name, shape, dt, kind="ExternalInput").ap()

    a = {}
    a["idx"] = din("idx", [T, 1], I32)
    a["emb"] = din("emb", [V, D])
    a["peT"] = din("peT", [DC, P, T], BF16)
    a["enc"] = din("enc", [T, D])
    a["masks"] = din("masks", [P, P], BF16)
    a["consts"] = din("consts", [P, NCONST])
    for nm in ("WqT", "WkT", "WvT", "Wo1T", "cWqT", "eWkT", "eWvT", "Wo2T"):
        a[nm] = din(nm, [D, D], MF)
    a["W1T"] = din("W1T", [D, FFN], MF)
    a["W2T"] = din("W2T", [FFN, D], MF)
    a["WoutT"] = din("WoutT", [D, V], BF16)
    # bias rows for row-major (V) projections
    a["bv_r"] = din("bv_r", [1, D], MF)
    a["ebv_r"] = din("ebv_r", [1, D], MF)
    out = nc.dram_tensor("out", [TOWN, V], OUT_DT, kind="ExternalOutput").ap()
    a["out"] = out

    with tile.TileContext(nc) as tc, \
         nc.allow_low_precision(reason="bf16 matmul operand pipeline"):
        _emit(tc, a)
    nc.compile()
    return nc


def _emit(tc, a):
    nc = tc.nc

    with tc.tile_pool(name="const", bufs=1) as cp, \
         tc.tile_pool(name="wqkv", bufs=1) as wqkv, \
         tc.tile_pool(name="wlate", bufs=1) as wlp, \
         tc.tile_pool(name="trunk", bufs=1) as trunkp:
        # ---- constants (computed on-chip; no DMA) ----
        ident = cp.tile([P, P], F32, tag="ident")
        make_identity(nc, ident[:])
        ident_b = cp.tile([P, P], BF16, tag="ident_b")
        nc.scalar.copy(ident_b[:], ident[:])
        zscr = cp.tile([P, TOWN], F32, tag="zscr")
        nc.vector.memset(zscr[:], 0.0)
        ones_col = cp.tile([P, 1], MF, tag="ones_col")
        nc.scalar.add(ones_col[:], zscr[:, 0:1], 1.0)
        ones_row = cp.tile([1, P], MF, tag="ones_row")
        nc.scalar.add(ones_row[:], zscr[0:1, 0:P], 1.0)
        eps_c = cp.tile([1, 1], F32, tag="eps_c")
        nc.vector.memset(eps_c[:], 1e-5)

        # ---- DMA issue order matters: idx first (gathers depend on it),
        # then QKV weights + pos encodings (startup critical path), then
        # the rest of the constants.
        idx_sb = cp.tile([P, 8], I32, tag="idx")
        nc.sync.dma_start(idx_sb[:],
                          a["idx"].rearrange("(c p) o -> p (c o)", p=P))
        peT_sb = cp.tile([P, DC, T], BF16, tag="pe")
        nc.sync.dma_start(peT_sb[:], a["peT"].rearrange("c p t -> p c t"))
        wk_all = wqkv.tile([P, DC, D], MF, tag="wk")
        nc.sync.dma_start(wk_all[:],
                          a["WkT"].rearrange("(c p) d -> p c d", p=P))
        wv_all = wqkv.tile([P, DC, D], MF, tag="wv")
        nc.sync.dma_start(wv_all[:],
                          a["WvT"].rearrange("(c p) d -> p c d", p=P))
        wq_all = wqkv.tile([P, DC, D], MF, tag="wq")
        nc.sync.dma_start(wq_all[:],
                          a["WqT"].rearrange("(c p) d -> p c d", p=P))
        consts = cp.tile([P, NCONST], F32, tag="consts")
        nc.sync.dma_start(consts[:], a["consts"][:, :])
        masks = cp.tile([P, P], BF16, tag="masks")
        nc.sync.dma_start(masks[:], a["masks"][:, :])
        # late trunk weights: issued up-front (fresh SBUF, no WAR stalls)
        ewk_all = wlp.tile([P, DC, D], MF, tag="ewk")
        nc.sync.dma_start(ewk_all[:],
                          a["eWkT"].rearrange("(c p) d -> p c d", p=P))
        ewv_all = wlp.tile([P, DC, D], MF, tag="ewv")
        nc.sync.dma_start(ewv_all[:],
                          a["eWvT"].rearrange("(c p) d -> p c d", p=P))
        wo1_all = wlp.tile([P, DC, D], MF, tag="wo1")
        nc.sync.dma_start(wo1_all[:],
                          a["Wo1T"].rearrange("(c p) d -> p c d", p=P))
        cwq_all = wlp.tile([P, DC, D], MF, tag="cwq")
        nc.sync.dma_start(cwq_all[:],
                          a["cWqT"].rearrange("(c p) d -> p c d", p=P))
        wo2_all = wlp.tile([P, DC, D], MF, tag="wo2")
        nc.sync.dma_start(wo2_all[:],
                          a["Wo2T"].rearrange("(c p) d -> p c d", p=P))
        w1_all = wlp.tile([P, DC, FFN], MF, tag="w1")
        nc.sync.dma_start(w1_all[:],
                          a["W1T"].rearrange("(c p) d -> p c d", p=P))
        w2_all = wlp.tile([P, FFN // P, D], MF, tag="w2")
        nc.sync.dma_start(w2_all[:],
                          a["W2T"].rearrange("(c p) d -> p c d", p=P))
        bv_r = cp.tile([1, D], MF, tag="bv_r")
        nc.sync.dma_start(bv_r[:], a["bv_r"][:, :])
        ebv_r = cp.tile([1, D], MF, tag="ebv_r")
        nc.sync.dma_start(ebv_r[:], a["ebv_r"][:, :])

        def cc(nm):
            o, n = _CB[nm]
            return consts[:, o:o + n]
        vmS, vmC = cc("vmS"), cc("vmC")
        bq_c, bk_c, bo1_c = cc("bq"), cc("bk"), cc("bo1")
        cbq_c, ebk_c, bo2_c = cc("cbq"), cc("ebk"), cc("bo2")
        b2_c, b1_c = cc("b2"), cc("b1")
        gc = {1: cc("gc1"), 2: cc("gc2"), 3: cc("gc3")}
        bc = {1: cc("bc1"), 2: cc("bc2"), 3: cc("bc3")}

        # ---- long-lived trunk activations ----
        x1T = [trunkp.tile([P, TOWN], MF, tag=f"x1T{c}", name=f"x1T{c}") for c in range(DC)]
        x2T = [trunkp.tile([P, TOWN], MF, tag=f"x2T{c}", name=f"x2T{c}") for c in range(DC)]

        # ================= shared helpers =================

        def proj_fm(dsts, src_halves, w_all, bias_col, func=AF.Identity,
                    pp=None, n_in=DC, psum_tag="proj"):
            """dsts[m][:, th*512:] = func(W @ src + b); feature-major halves.
            w_all: [P, n_in, len(dsts)*P] preloaded weight tile."""
            for th in range(len(src_halves)):
                for m in range(len(dsts)):
                    ps = pp.tile([P, 512], F32, tag=psum_tag)
                    for c in range(n_in):
                        nc.tensor.matmul(
                            ps[:],
                            lhsT=w_all[:, c, ts(m, P)],
                            rhs=src_halves[th][c][:, :],
                            start=(c == 0), stop=(c == n_in - 1))
                    nc.scalar.activation(
                        dsts[m][:, th * 512:(th + 1) * 512], ps[:],
                        func, bias=bias_col[:, m: m + 1], scale=1.0)

        def vproj_tile(vt, ps, vmcol, use_act):
            """vt rows scaled by the 0/1 key-validity column vmcol; the
            interleaved 65th columns get vmcol itself (denominator trick).
            ACT during QKV (DVE paces x0 writes there); DVE during
            attention windows (ACT saturated by Exps)."""
            v3 = vt[:].rearrange("p (h e) -> p h e", e=65)
            ps3 = ps[:].rearrange("p (h e) -> p h e", e=64)
            z3 = zscr[:, 0:8].rearrange("p (h e) -> p h e", e=1)
            if use_act:
                nc.scalar.activation(v3[:, :, 0:64], ps3, AF.Copy,
                                     bias=0.0, scale=vmcol)
                nc.scalar.activation(v3[:, :, 64:65], z3, AF.Identity,
                                     bias=vmcol, scale=1.0)
            else:
                nc.vector.tensor_scalar(v3[:, :, 0:64], ps3, vmcol, None,
                                        op0=OP.mult)
                nc.vector.tensor_scalar(v3[:, :, 64:65], z3, vmcol, None,
                                        op0=OP.add)

        def vproj(vtiles, src_slice, w_all, bias_row, vmask, pp=None,
                  psum_tag="vproj", tok_range=None, use_act=False):
            """Row-major V projection with interleaved ones columns.

            vtiles[t]: [P, H*65]; cols h*65..h*65+63 = V features of head h,
            col h*65+64 = 1.0 (softmax-denominator trick). Rows of masked
            keys are zeroed via the per-partition 0/1 column vmask[:, t]
            (folds the key-padding bias out of the Exp)."""
            for t in tok_range if tok_range is not None else range(len(vtiles)):
                ps = pp.tile([P, D], F32, tag=psum_tag, name="vps")
                for c in range(DC):
                    nc.tensor.matmul(ps[:], lhsT=src_slice(c, t),
                                     rhs=w_all[:, c, :],
                                     start=(c == 0), stop=False)
                nc.tensor.matmul(ps[:], lhsT=ones_row[:], rhs=bias_row[:],
                                 start=False, stop=True)
                vproj_tile(vtiles[t], ps, vmask[:, t: t + 1], use_act)

        def attention(kT, vtiles, qT, use_masks, mergedT, pools, spbufs=3):
            """Per head: all paired score matmuls first, then the AV
            accumulation, then denominator broadcast + merge.
            Key-padding is folded into zeroed V rows, so the Exp bias is 0."""
            sp, avp, rp, sbp = pools

            def emit_scores(h):
                hc, off = h // 2, (h % 2) * DKH
                pts = []
                for pr in range(4):
                    s = sp.tile([P, 2, TOWN], F32, tag="s", bufs=spbufs)
                    for i in range(2):
                        kc = 2 * pr + i
                        nc.tensor.matmul(
                            s[:, i, :], lhsT=kT[hc][off:off + DKH, ts(kc, P)],
                            rhs=qT[hc][off:off + DKH, :], start=True,
                            stop=True)
                    pt = sbp.tile([P, 2, TOWN], MF, tag="pT", bufs=4)
                    nc.scalar.activation(pt[:], s[:], AF.Exp,
                                         bias=0.0, scale=0.125)
                    if use_masks and pr >= 2:
                        # causal triangle on the diagonal 128x128 block;
                        # columns left of it are never read by the AV below
                        for i in range(2):
                            kc = 2 * pr + i
                            q0 = (kc - 4) * P
                            nc.vector.tensor_tensor(
                                pt[:, i, q0:q0 + P], pt[:, i, q0:q0 + P],
                                masks[:, :], op=OP.mult)
                    pts.append(pt)
                return pts

            def emit_avs(h, pts):
                av = avp.tile([DKH + 1, TOWN], F32, tag="av")
                for kc in range(8):
                    q0 = (kc - 4) * P if use_masks and kc >= 4 else 0
                    nc.tensor.matmul(av[:, q0:],
                                     lhsT=vtiles[kc][:, h * 65: h * 65 + 65],
                                     rhs=pts[kc // 2][:, kc % 2, q0:],
                                     start=(kc == 0), stop=(kc == 7),
                                     skip_group_check=True)
                srow = sbp.tile([1, TOWN], MF, tag="srow", bufs=2)
                nc.scalar.copy(srow[:], av[DKH: DKH + 1, :])
                return av, srow

            def emit_rmerge(h, av, srow):
                hc, off = h // 2, (h % 2) * DKH
                R = rp.tile([DKH, TOWN], F32, tag="R")
                nc.tensor.matmul(R[:], lhsT=ones_row[:, 0:DKH],
                                 rhs=srow[:], start=True, stop=True)
                rinv = sbp.tile([DKH, TOWN], F32, tag="rinv", bufs=2)
                nc.vector.reciprocal_approx_fast(rinv[:], R[:])
                nc.vector.tensor_tensor(mergedT[hc][off:off + DKH, :],
                                        av[0:DKH, :], rinv[:], op=OP.mult)

            for h in range(H):
                pts = emit_scores(h)
                av, srow = emit_avs(h, pts)
                emit_rmerge(h, av, srow)

        def layernorm(srcs, i, dsts, pools, filler=None, pad_pool=None,
                      pads=(8, 10)):
            """dsts = LN(srcs) with gain/bias pack i (feature-major chunks).
            `filler` emits independent PE work between the stat matmuls and
            the broadcast matmuls so the PE never dips while ACT/DVE compute
            the row statistics (a PE dip triggers a hardware k=4 window)."""
            statp, bcp, sbp = pools
            ssum = statp.tile([1, TOWN], F32, tag="ssum")
            ssq = statp.tile([1, TOWN], F32, tag="ssq")
            for c in range(DC):
                nc.tensor.matmul(ssum[:], lhsT=ones_col[:], rhs=srcs[c][:],
                                 start=(c == 0), stop=(c == DC - 1))
            for c in range(DC):
                sq = sbp.tile([P, TOWN], MF, tag="sq", bufs=2)
                nc.scalar.square(sq[:], srcs[c][:])
                nc.tensor.matmul(ssq[:], lhsT=ones_col[:], rhs=sq[:],
                                 start=(c == 0), stop=(c == DC - 1))
            mu = sbp.tile([1, TOWN], MF, tag="row", bufs=3, name="mu")
            nc.scalar.mul(mu[:], ssum[:], 1.0 / D)
            musq = sbp.tile([1, TOWN], F32, tag="row", bufs=3, name="musq")
            nc.vector.scalar_tensor_tensor(musq[:], in0=mu[:], scalar=1.0,
                                           in1=mu[:], op0=OP.mult, op1=OP.mult)
            var = sbp.tile([1, TOWN], F32, tag="row", bufs=3, name="var")
            nc.vector.scalar_tensor_tensor(var[:], in0=ssq[:], scalar=1.0 / D,
                                           in1=musq[:], op0=OP.mult,
                                           op1=OP.subtract)
            std = sbp.tile([1, TOWN], MF, tag="row", bufs=3, name="std")
            nc.scalar.activation(std[:], var[:], AF.Sqrt, bias=eps_c[:], scale=1.0)
            def pad(n):
                if pad_pool is None or n == 0:
                    return
                pt = pad_pool.tile([P, TOWN], F32, tag="proj")
                for _ in range(n):
                    nc.tensor.matmul(pt[:, 0:P], lhsT=ones_row[:],
                                     rhs=masks[0:1, 0:P],
                                     start=True, stop=True)
            if filler is not None:
                filler()
            else:
                pad(pads[0])
            mu_b = bcp.tile([P, TOWN], F32, tag="mu_b", bufs=1)
            nc.tensor.matmul(mu_b[:], lhsT=ones_row[:], rhs=mu[:],
                             start=True, stop=True)
            std_b = bcp.tile([P, TOWN], F32, tag="std_b", bufs=1)
            nc.tensor.matmul(std_b[:], lhsT=ones_row[:], rhs=std[:],
                             start=True, stop=True)
            ainv = sbp.tile([P, TOWN], F32, tag="ainv", bufs=1)
            nc.vector.reciprocal_approx_fast(ainv[:], std_b[:])
            pad(pads[1])
            for c in range(DC):
                t1 = sbp.tile([P, TOWN], F32, tag="lnt", bufs=2)
                nc.vector.tensor_tensor(t1[:], srcs[c][:], mu_b[:], op=OP.subtract)
                t2 = sbp.tile([P, TOWN], F32, tag="lnt2", bufs=2)
                nc.vector.tensor_tensor(t2[:], t1[:], ainv[:], op=OP.mult)
                nc.scalar.activation(dsts[c][:], t2[:], AF.Identity,
                                     bias=bc[i][:, c: c + 1],
                                     scale=gc[i][:, c: c + 1])

        # ====== blocks A+B: embed, self-attn (overlapped with enc/ek/ev), =====
        # ====== LN1, cross-attn, LN2                                       =====
        with tc.tile_pool(name="blkB", bufs=1) as bB:
            ekT = [bB.tile([P, T], MF, tag=f"ekT{c}", name=f"ekT{c}") for c in range(DC)]
            evsb = [bB.tile([P, H * 65], MF, tag=f"ev{t}", name=f"ev{t}") for t in range(8)]
            cqT = [bB.tile([P, TOWN], MF, tag=f"cqT{c}", name=f"cqT{c}") for c in range(DC)]
            mergedT2 = [bB.tile([P, TOWN], MF, tag=f"mg2T{c}", name=f"mg2T{c}") for c in range(DC)]

            encp_cm = tc.tile_pool(name="encp", bufs=4)
            encp = encp_cm.__enter__()

            enc_tiles = {}

            def emit_enc_load(th, mp):
                """Encoder half th: load + transpose enc tokens (PE filler)."""
                encTh = []
                for c in range(DC):
                    e = encp.tile([P, TOWN], MF, tag="encTh",
                                  name=f"encTh{th}{c}")
                    encTh.append(e)
                es_all = encp.tile([P, 4, D], F32, tag="es_all",
                                   name=f"es{th}", bufs=2)
                nc.sync.dma_start(
                    es_all[:],
                    a["enc"].rearrange("(g t p) d -> p (g t) d",
                                       g=2, p=P)[:, th * 4:(th + 1) * 4, :])
                for t in range(4):
                    for c in range(DC):
                        tp = mp.tile([P, P], F32, tag="misc", name="tp")
                        nc.tensor.transpose(tp[:], es_all[:, t, ts(c, P)],
                                            ident[:])
                        nc.vector.tensor_copy(encTh[c][:, ts(t, P)], tp[:])
                enc_tiles[th] = encTh

            def emit_enc_proj(th, mp):
                """Encoder half th: eK / eV projections."""
                encTh = enc_tiles.pop(th)
                for m in range(DC):
                    ps = mp.tile([P, TOWN], F32, tag="misc", name="ekp")
                    for c in range(DC):
                        nc.tensor.matmul(
                            ps[:], lhsT=ewk_all[:, c, ts(m, P)],
                            rhs=encTh[c][:, :],
                            start=(c == 0), stop=(c == DC - 1))
                    nc.vector.tensor_scalar(
                        ekT[m][:, th * 512:(th + 1) * 512], ps[:],
                        ebk_c[:, m: m + 1], None, op0=OP.add)
                vproj(evsb, lambda c, t: encTh[c][:, ts(t - th * 4, P)],
                      ewv_all, ebv_r, vmC, pp=mp, psum_tag="misc",
                      tok_range=range(th * 4, th * 4 + 4))

            with tc.tile_pool(name="blkA", bufs=1) as bA, \
                 tc.tile_pool(name="rotA", bufs=3) as rA:
                x0p = [bA.tile([P, TOWN], MF, tag=f"x0p{c}", name=f"x0p{c}") for c in range(DC)]
                x0o = [bA.tile([P, TOWN], MF, tag=f"x0o{c}", name=f"x0o{c}") for c in range(DC)]
                kT = [bA.tile([P, T], MF, tag=f"kT{c}", name=f"kT{c}") for c in range(DC)]
                vsb = [bA.tile([P, H * 65], MF, tag=f"v{t}", name=f"v{t}") for t in range(8)]
                qT = [bA.tile([P, TOWN], MF, tag=f"qT{c}", name=f"qT{c}") for c in range(DC)]
                mergedT = [bA.tile([P, TOWN], MF, tag=f"mgT{c}", name=f"mgT{c}") for c in range(DC)]

                def x0slice(c, t):
                    return (x0p[c][:, ts(t, P)] if t < 4
                            else x0o[c][:, ts(t - 4, P)])

                # --- embedding gather + transpose + scale + pos encoding ---
                with tc.tile_pool(name="psA0", bufs=3, space="PSUM") as pp0:
                    for t in range(8):
                        xg = rA.tile([P, D], F32, tag="xg", bufs=8)
                        nc.gpsimd.indirect_dma_start(
                            out=xg[:], out_offset=None, in_=a["emb"][:, :],
                            in_offset=bass.IndirectOffsetOnAxis(
                                ap=idx_sb[:, t: t + 1], axis=0))
                        for c in range(DC):
                            tp = pp0.tile([P, P], F32, tag="tp")
                            nc.tensor.transpose(tp[:], xg[:, ts(c, P)], ident[:])
                            nc.vector.scalar_tensor_tensor(
                                x0slice(c, t), in0=tp[:], scalar=SQRT_D,
                                in1=peT_sb[:, c, ts(t, P)],
                                op0=OP.mult, op1=OP.add)

                # --- K, V, Q projections ---
                with tc.tile_pool(name="psA1", bufs=3, space="PSUM") as pp1:
                    proj_fm(kT, [x0p, x0o], wk_all, bk_c, pp=pp1)
                    vproj(vsb, x0slice, wv_all, bv_r, vmS, pp=pp1,
                          use_act=True)
                    proj_fm(qT, [x0o], wq_all, bq_c, pp=pp1)

                # --- self-attention overlapped with enc transpose + ek/ev ---
                with tc.tile_pool(name="psS", bufs=2, space="PSUM") as sp, \
                     tc.tile_pool(name="psAV", bufs=2, space="PSUM") as avp, \
                     tc.tile_pool(name="psR", bufs=1, space="PSUM") as rp, \
                     tc.tile_pool(name="psMisc", bufs=1, space="PSUM") as mp, \
                     tc.tile_pool(name="sbA", bufs=3) as sbp:
                    attention(kT, vsb, qT, True, mergedT,
                              (sp, avp, rp, sbp), spbufs=2)
                    # gap filler: encoder half 0 transpose + eK/eV projections
                    emit_enc_load(0, mp)
                    emit_enc_proj(0, mp)

                # --- Wo1 + residual + LN1 -> x1T (enc half 1 fills LN1) ---
                with tc.tile_pool(name="psA2", bufs=2, space="PSUM") as pp2, \
                     tc.tile_pool(name="psStat", bufs=1, space="PSUM") as statp, \
                     tc.tile_pool(name="psBC", bufs=2, space="PSUM") as bcp, \
                     tc.tile_pool(name="psMisc1", bufs=2, space="PSUM") as mp1, \
                     tc.tile_pool(name="sbLN", bufs=3) as sbp:
                    ln_in = []
                    for m in range(DC):
                        ps = pp2.tile([P, TOWN], F32, tag="proj")
                        for c in range(DC):
                            nc.tensor.matmul(ps[:], lhsT=wo1_all[:, c, ts(m, P)],
                                             rhs=mergedT[c][:],
                                             start=(c == 0), stop=(c == DC - 1))
                        li = sbp.tile([P, TOWN], MF, tag="li", bufs=4,
                                      name=f"li{m}")
                        nc.vector.scalar_tensor_tensor(
                            li[:], in0=ps[:], scalar=bo1_c[:, m: m + 1],
                            in1=x0o[m][:], op0=OP.add, op1=OP.add)
                        ln_in.append(li)
                    layernorm(ln_in, 1, x1T, (statp, bcp, sbp),
                              filler=lambda: emit_enc_load(1, mp1))
                    emit_enc_proj(1, mp1)

            # --- cross-attention ---
            with tc.tile_pool(name="psB1", bufs=3, space="PSUM") as pp1:
                proj_fm(cqT, [x1T], cwq_all, cbq_c, pp=pp1)
            encp_cm.__exit__(None, None, None)

            with tc.tile_pool(name="psS", bufs=2, space="PSUM") as sp, \
                 tc.tile_pool(name="psAV", bufs=2, space="PSUM") as avp, \
                 tc.tile_pool(name="psR", bufs=2, space="PSUM") as rp, \
                 tc.tile_pool(name="sbB", bufs=3) as sbp:
                attention(ekT, evsb, cqT, False, mergedT2,
                          (sp, avp, rp, sbp), spbufs=2)

            with tc.tile_pool(name="psB2", bufs=2, space="PSUM") as pp2, \
                 tc.tile_pool(name="psStat", bufs=1, space="PSUM") as statp, \
                 tc.tile_pool(name="psBC", bufs=2, space="PSUM") as bcp, \
                 tc.tile_pool(name="sbLN", bufs=3) as sbp:
                ln_in = []
                for m in range(DC):
                    ps = pp2.tile([P, TOWN], F32, tag="proj")
                    for c in range(DC):
                        nc.tensor.matmul(ps[:], lhsT=wo2_all[:, c, ts(m, P)],
                                         rhs=mergedT2[c][:],
                                         start=(c == 0), stop=(c == DC - 1))
                    li = sbp.tile([P, TOWN], MF, tag=f"li{m}", name=f"li{m}",
                                  bufs=1)
                    nc.vector.scalar_tensor_tensor(
                        li[:], in0=ps[:], scalar=bo2_c[:, m: m + 1],
                        in1=x1T[m][:], op0=OP.add, op1=OP.add)
                    ln_in.append(li)
                layernorm(ln_in, 2, x2T, (statp, bcp, sbp), pad_pool=pp2,
                          pads=(42, 28))

        # ================= blocks C+D =================
        with tc.tile_pool(name="late", bufs=1) as latep, \
             tc.tile_pool(name="wD", bufs=3) as wdp:
          x3T = [latep.tile([P, TOWN], MF, tag=f"x3T{c}", name=f"x3T{c}")
                 for c in range(DC)]

          wout_tiles = {}

          def wout_tile(vg):
              if vg in wout_tiles or vg >= NVG:
                  return wout_tiles.get(vg)
              w_all = wdp.tile([P, DC, VG * VCH], BF16, tag="wo")
              nc.sync.dma_start(
                  w_all[:],
                  a["WoutT"].rearrange("(c p) v -> p c v", p=P)
                  [:, :, vg * VG * VCH:(vg + 1) * VG * VCH])
              wout_tiles[vg] = w_all
              return w_all

          # ----- block C: FFN, LN3 -----
          with tc.tile_pool(name="hC", bufs=1) as hp, \
               tc.tile_pool(name="psC", bufs=4, space="PSUM") as pp, \
               tc.tile_pool(name="psStat", bufs=1, space="PSUM") as statp, \
               tc.tile_pool(name="psBC", bufs=2, space="PSUM") as bcp, \
               tc.tile_pool(name="sbC", bufs=3) as sbp:
              # prefetch the first two vocab weight groups during the FFN
              wout_tile(0)
              wout_tile(1)
              hT = [hp.tile([P, TOWN], MF, tag=f"hT{m}", name=f"hT{m}") for m in range(FFN // P)]
              for w in range(2):
                  wps = [pp.tile([P, 512], F32, tag="proj",
                                 name=f"w1ps{w}{j}") for j in range(4)]
                  for c in range(DC):
                      for j in range(4):
                          m = 4 * w + j
                          nc.tensor.matmul(wps[j][:],
                                           lhsT=w1_all[:, c, ts(m, P)],
                                           rhs=x2T[c][:],
                                           start=(c == 0), stop=(c == DC - 1))
                  for j in range(4):
                      m = 4 * w + j
                      nc.scalar.activation(hT[m][:], wps[j][:], AF.Relu,
                                           bias=b1_c[:, m: m + 1], scale=1.0)
              ln_in = []
              for m in range(DC):
                  ps = pp.tile([P, TOWN], F32, tag="proj")
                  for c in range(FFN // P):
                      nc.tensor.matmul(ps[:], lhsT=w2_all[:, c, ts(m, P)],
                                       rhs=hT[c][:],
                                       start=(c == 0), stop=(c == FFN // P - 1))
                  li = sbp.tile([P, TOWN], MF, tag=f"li{m}", name=f"li{m}", bufs=1)
                  nc.vector.scalar_tensor_tensor(
                      li[:], in0=ps[:], scalar=b2_c[:, m: m + 1], in1=x2T[m][:],
                      op0=OP.add, op1=OP.add)
                  ln_in.append(li)
              layernorm(ln_in, 3, x3T, (statp, bcp, sbp), pad_pool=pp,
                        pads=(42, 36))

          # ================= block D: vocab projection =================
          with tc.tile_pool(name="stD", bufs=6) as stp, \
               tc.tile_pool(name="psD", bufs=2, space="PSUM") as pp:
              outr = a["out"].rearrange("(t p) v -> p t v", p=P)
              for vg in range(NVG):
                  w_all = wout_tile(vg)
                  wout_tile(vg + 1)                  # rolling prefetch
                  del wout_tiles[vg]
                  for t in range(TOWN // P):
                      ps = pp.tile([P, VG, 512], F32, tag="vps")
                      for j in range(VG):
                          for c in range(DC):
                              nc.tensor.matmul(
                                  ps[:, j, 0:VCH],
                                  lhsT=x3T[c][:, ts(t, P)],
                                  rhs=w_all[:, c, ts(j, VCH)],
                                  start=(c == 0), stop=(c == DC - 1))
                      stage = stp.tile([P, VG * VCH], OUT_DT, tag="stage")
                      st3 = stage[:].rearrange("p (j e) -> p j e", e=VCH)
                      if t % 2 == 0:
                          nc.scalar.copy(st3, ps[:, :, 0:VCH])
                      else:
                          nc.vector.tensor_copy(st3, ps[:, :, 0:VCH])
                      nc.sync.dma_start(
                          outr[:, t, vg * VG * VCH:(vg + 1) * VG * VCH],
                          stage[:])


# --------------------------------------------------------------------------
# host-side input preparation
# --------------------------------------------------------------------------

def _pos_encoding_np(t, d):
    pos = np.arange(t, dtype=np.float32)[:, None]
    freqs = 1.0 / (10000.0 ** (np.arange(0, d, 2, dtype=np.float32) / d))
    pe = np.zeros((t, d), np.float32)
    pe[:, 0::2] = np.sin(pos * freqs)
    pe[:, 1::2] = np.cos(pos * freqs)
    return pe


def _col_pack(b):
    """[n] -> [P, n//P] with element (p, c) = b[c*P + p]."""
    b = np.asarray(b, np.float32)
    return np.ascontiguousarray(b.reshape(-1, P).T)


def prep_in_maps(inputs):
    import ml_dtypes
    bf16 = ml_dtypes.bfloat16
    gi = lambda n: np.asarray(inputs[n])
    tokens = gi("tokens").astype(np.int32)                      # [4, 1024]
    enc_all = np.ascontiguousarray(gi("enc_embeddings").astype(np.float32))
    enc_pad = gi("enc_pad_mask").astype(bool)
    emb = np.ascontiguousarray(gi("emb").astype(np.float32))

    shared = {"emb": emb}
    for nm in ("Wq", "Wk", "Wv", "Wo1", "cWq", "eWk", "eWv", "Wo2", "W1", "W2",
               "Wout"):
        shared[nm + "T"] = np.ascontiguousarray(
            gi(nm).astype(np.float32).T).astype(bf16)
    shared["bv_r"] = gi("bv").astype(np.float32).reshape(1, D).astype(bf16)
    shared["ebv_r"] = gi("ebv").astype(np.float32).reshape(1, D).astype(bf16)

    # packed fp32 constant block (per-core biasS/biasC patched below)
    cblk = np.zeros((P, NCONST), np.float32)
    def setc(nm, arr):
        o, n = _CB[nm]
        cblk[:, o:o + n] = arr
    for nm, src in (("bq", "bq"), ("bk", "bk"), ("bo1", "bo1"), ("cbq", "cbq"),
                    ("ebk", "ebk"), ("bo2", "bo2"), ("b2", "b2"), ("b1", "b1")):
        setc(nm, _col_pack(gi(src)))
    for i, (g, b) in ((1, ("g1", "be1")), (2, ("g3", "be3")), (3, ("g2", "be2"))):
        setc(f"gc{i}", _col_pack(gi(g)))
        setc(f"bc{i}", _col_pack(gi(b)))

    # causal 0/1 lower-triangle for the diagonal 128-blocks (key <= query)
    kk = np.arange(P)[:, None]
    qq = np.arange(P)[None, :]
    shared["masks"] = np.where(kk <= qq, 1.0, 0.0).astype(bf16)

    pe = _pos_encoding_np(T, D)

    in_maps = []
    for core in range(8):
        b, hf = core // 2, core % 2
        own = tokens[b, hf * 512:(hf + 1) * 512]
        idx_full = np.concatenate([tokens[b, :512], own])        # [1024]
        pe_slots = np.concatenate([pe[:512], pe[hf * 512:(hf + 1) * 512]], axis=0)
        peT = np.ascontiguousarray(
            pe_slots.T.reshape(DC, P, T, order="C"))
        # pe_slots.T is [D, T]; reshape to [DC, P, T] splits D into chunks
        vmS = np.where(idx_full == PAD_ID, 0.0, 1.0).astype(np.float32)
        if hf == 0:
            vmS[:512] = 0.0                                      # no prefix half
        vmC = np.where(enc_pad[b], 0.0, 1.0).astype(np.float32)
        m = dict(shared)
        m["idx"] = np.ascontiguousarray(idx_full.reshape(T, 1))
        m["peT"] = peT.astype(bf16)
        m["enc"] = np.ascontiguousarray(enc_all[b])
        cb = cblk.copy()
        cb[:, _CB["vmS"][0]:_CB["vmS"][0] + 8] = \
            np.ascontiguousarray(vmS.reshape(8, P).T)
        cb[:, _CB["vmC"][0]:_CB["vmC"][0] + 8] = \
            np.ascontiguousarray(vmC.reshape(8, P).T)
        m["consts"] = cb
        in_maps.append(m)
    return in_maps


def assemble(results, inputs):
    full = np.empty((4, 1024, V), np.float32)
    for core in range(8):
        b, hf = core // 2, core % 2
        full[b, hf * 512:(hf + 1) * 512] = np.asarray(
            results[core]["out"]).astype(np.float32)
    bout = np.asarray(inputs["bout"], np.float32)
    if np.any(bout):
        full += bout[None, None, :]
    return full


# --------------------------------------------------------------------------
# public entry point
# --------------------------------------------------------------------------

def kernel(**inputs):
    from concourse.bass_utils import run_bass_kernel_spmd
    nc = build_module()
    in_maps = prep_in_maps(inputs)
    res = run_bass_kernel_spmd(nc, in_maps, core_ids=list(range(8)))
    return assemble(res.results, inputs)


if __name__ == "__main__":
    nc = build_module()
    print("built ok")


# revision 32
# speedup vs baseline: 1.0605x; 1.0605x over previous
"""Trainium2 Bass kernel for nn_DecoderTrans (dense transformer decoder layer + vocab head).

Sharding: 8 cores = (batch b, half hf). Each core computes the full trunk for its
512 "own" tokens (queries) and the K/V context for the whole 1024-token sequence
of its batch element. Own tokens always occupy key slots [512, 1024) so the
program is uniform SPMD; per-core mask/bias DATA encodes the causal structure.
Activations are kept feature-major (x^T: [D, tokens]) throughout; weights are
shipped pre-transposed ([d_in, d_out]) in bf16 (PSUM accumulation stays fp32).
"""
import math
import os
import sys

sys.path.insert(0, "/opt/trn_rl_repo")

import numpy as np

import concourse.bass as bass
import concourse.tile as tile
from concourse import bacc, mybir
from concourse.bass import ts
from concourse.masks import make_identity

P = 128
D = 512
DC = D // P          # 4 feature chunks
T = 1024             # full sequence (keys)
TOWN = 512           # own tokens per core (queries), slots [512, 1024)
H = 8
DKH = 64             # head dim
V = 32000
VCH = 500            # vocab columns per matmul (fits PSUM bank)
VG = 4               # vocab chunks per group
NVG = V // (VCH * VG)  # 16 groups
FFN = 2 * D
NEG = -30000.0
SQRT_D = math.sqrt(D)
PAD_ID = 0

F32 = mybir.dt.float32
I32 = mybir.dt.int32
AF = mybir.ActivationFunctionType
OP = mybir.AluOpType

BF16 = mybir.dt.bfloat16
MF = BF16            # matmul operand dtype (activations + weights)
OUT_DT = BF16        # output staging dtype (halves the 32MB/core logit writeback)

# packed fp32 constant block column offsets: biasS(8) biasC(8) then
# per-projection bias columns and layernorm gain/bias packs
_CB = {}
_off = 0
for _nm, _n in (("vmS", 8), ("vmC", 8), ("bq", DC), ("bk", DC),
                ("bo1", DC), ("cbq", DC), ("ebk", DC), ("bo2", DC),
                ("b2", DC), ("b1", FFN // P),
                ("gc1", DC), ("bc1", DC), ("gc2", DC), ("bc2", DC),
                ("gc3", DC), ("bc3", DC)):
    _CB[_nm] = (_off, _n)
    _off += _n
NCONST = _off


# --------------------------------------------------------------------------
# program builder
# --------------------------------------------------------------------------

def build_module():
    nc = bacc.Bacc("TRN2", target_bir_lowering=False, debug=False)

    def din(name, shape, dt=F32):
        return nc.dram_tensor(name, shape, dt, kind="ExternalInput").ap()

    a = {}
    a["idx"] = din("idx", [T, 1], I32)
    a["emb"] = din("emb", [V, D])
    a["peT"] = din("peT", [DC, P, T], BF16)
    a["enc"] = din("enc", [T, D])
    a["masks"] = din("masks", [P, P], BF16)
    a["consts"] = din("consts", [P, NCONST])
    for nm in ("WqT", "WkT", "WvT", "Wo1T", "cWqT", "eWkT", "eWvT", "Wo2T"):
        a[nm] = din(nm, [D, D], MF)
    a["W1T"] = din("W1T", [D, FFN], MF)
    a["W2T"] = din("W2T", [FFN, D], MF)
    a["WoutT"] = din("WoutT", [D, V], BF16)
    # bias rows for row-major (V) projections
    a["bv_r"] = din("bv_r", [1, D], MF)
    a["ebv_r"] = din("ebv_r", [1, D], MF)
    out = nc.dram_tensor("out", [TOWN, V], OUT_DT, kind="ExternalOutput").ap()
    a["out"] = out

    with tile.TileContext(nc) as tc, \
         nc.allow_low_precision(reason="bf16 matmul operand pipeline"):
        _emit(tc, a)
    nc.compile()
    return nc


def _emit(tc, a):
    nc = tc.nc

    with tc.tile_pool(name="const", bufs=1) as cp, \
         tc.tile_pool(name="wqkv", bufs=1) as wqkv, \
         tc.tile_pool(name="wlate", bufs=1) as wlp, \
         tc.tile_pool(name="trunk", bufs=1) as trunkp:
        # ---- constants (computed on-chip; no DMA) ----
        ident = cp.tile([P, P], F32, tag="ident")
        make_identity(nc, ident[:])
        ident_b = cp.tile([P, P], BF16, tag="ident_b")
        nc.scalar.copy(ident_b[:], ident[:])
        zscr = cp.tile([P, TOWN], F32, tag="zscr")
        nc.vector.memset(zscr[:], 0.0)
        ones_col = cp.tile([P, 1], MF, tag="ones_col")
        nc.scalar.add(ones_col[:], zscr[:, 0:1], 1.0)
        ones_row = cp.tile([1, P], MF, tag="ones_row")
        nc.scalar.add(ones_row[:], zscr[0:1, 0:P], 1.0)
        eps_c = cp.tile([1, 1], F32, tag="eps_c")
        nc.vector.memset(eps_c[:], 1e-5)

        # ---- DMA issue order matters: idx first (gathers depend on it),
        # then QKV weights + pos encodings (startup critical path), then
        # the rest of the constants.
        idx_sb = cp.tile([P, 8], I32, tag="idx")
        nc.sync.dma_start(idx_sb[:],
                          a["idx"].rearrange("(c p) o -> p (c o)", p=P))
        peT_sb = cp.tile([P, DC, T], BF16, tag="pe")
        nc.sync.dma_start(peT_sb[:], a["peT"].rearrange("c p t -> p c t"))
        wk_all = wqkv.tile([P, DC, D], MF, tag="wk")
        nc.sync.dma_start(wk_all[:],
                          a["WkT"].rearrange("(c p) d -> p c d", p=P))
        wv_all = wqkv.tile([P, DC, D], MF, tag="wv")
        nc.sync.dma_start(wv_all[:],
                          a["WvT"].rearrange("(c p) d -> p c d", p=P))
        wq_all = wqkv.tile([P, DC, D], MF, tag="wq")
        nc.sync.dma_start(wq_all[:],
                          a["WqT"].rearrange("(c p) d -> p c d", p=P))
        consts = cp.tile([P, NCONST], F32, tag="consts")
        nc.sync.dma_start(consts[:], a["consts"][:, :])
        masks = cp.tile([P, P], BF16, tag="masks")
        nc.sync.dma_start(masks[:], a["masks"][:, :])
        # late trunk weights: issued up-front (fresh SBUF, no WAR stalls)
        ewk_all = wlp.tile([P, DC, D], MF, tag="ewk")
        nc.sync.dma_start(ewk_all[:],
                          a["eWkT"].rearrange("(c p) d -> p c d", p=P))
        ewv_all = wlp.tile([P, DC, D], MF, tag="ewv")
        nc.sync.dma_start(ewv_all[:],
                          a["eWvT"].rearrange("(c p) d -> p c d", p=P))
        wo1_all = wlp.tile([P, DC, D], MF, tag="wo1")
        nc.sync.dma_start(wo1_all[:],
                          a["Wo1T"].rearrange("(c p) d -> p c d", p=P))
        cwq_all = wlp.tile([P, DC, D], MF, tag="cwq")
        nc.sync.dma_start(cwq_all[:],
                          a["cWqT"].rearrange("(c p) d -> p c d", p=P))
        wo2_all = wlp.tile([P, DC, D], MF, tag="wo2")
        nc.sync.dma_start(wo2_all[:],
                          a["Wo2T"].rearrange("(c p) d -> p c d", p=P))
        w1_all = wlp.tile([P, DC, FFN], MF, tag="w1")
        nc.sync.dma_start(w1_all[:],
                          a["W1T"].rearrange("(c p) d -> p c d", p=P))
        w2_all = wlp.tile([P, FFN // P, D], MF, tag="w2")
        nc.sync.dma_start(w2_all[:],
                          a["W2T"].rearrange("(c p) d -> p c d", p=P))
        bv_r = cp.tile([1, D], MF, tag="bv_r")
        nc.sync.dma_start(bv_r[:], a["bv_r"][:, :])
        ebv_r = cp.tile([1, D], MF, tag="ebv_r")
        nc.sync.dma_start(ebv_r[:], a["ebv_r"][:, :])

        def cc(nm):
            o, n = _CB[nm]
            return consts[:, o:o + n]
        vmS, vmC = cc("vmS"), cc("vmC")
        bq_c, bk_c, bo1_c = cc("bq"), cc("bk"), cc("bo1")
        cbq_c, ebk_c, bo2_c = cc("cbq"), cc("ebk"), cc("bo2")
        b2_c, b1_c = cc("b2"), cc("b1")
        gc = {1: cc("gc1"), 2: cc("gc2"), 3: cc("gc3")}
        bc = {1: cc("bc1"), 2: cc("bc2"), 3: cc("bc3")}

        # ---- long-lived trunk activations ----
        x1T = [trunkp.tile([P, TOWN], MF, tag=f"x1T{c}", name=f"x1T{c}") for c in range(DC)]
        x2T = [trunkp.tile([P, TOWN], MF, tag=f"x2T{c}", name=f"x2T{c}") for c in range(DC)]

        # ================= shared helpers =================

        def proj_fm(dsts, src_halves, w_all, bias_col, func=AF.Identity,
                    pp=None, n_in=DC, psum_tag="proj"):
            """dsts[m][:, th*512:] = func(W @ src + b); feature-major halves.
            w_all: [P, n_in, len(dsts)*P] preloaded weight tile."""
            for th in range(len(src_halves)):
                for m in range(len(dsts)):
                    ps = pp.tile([P, 512], F32, tag=psum_tag)
                    for c in range(n_in):
                        nc.tensor.matmul(
                            ps[:],
                            lhsT=w_all[:, c, ts(m, P)],
                            rhs=src_halves[th][c][:, :],
                            start=(c == 0), stop=(c == n_in - 1))
                    nc.scalar.activation(
                        dsts[m][:, th * 512:(th + 1) * 512], ps[:],
                        func, bias=bias_col[:, m: m + 1], scale=1.0)

        def vproj_tile(vt, ps, vmcol, use_act):
            """vt rows scaled by the 0/1 key-validity column vmcol; the
            interleaved 65th columns get vmcol itself (denominator trick).
            ACT during QKV (DVE paces x0 writes there); DVE during
            attention windows (ACT saturated by Exps)."""
            v3 = vt[:].rearrange("p (h e) -> p h e", e=65)
            ps3 = ps[:].rearrange("p (h e) -> p h e", e=64)
            z3 = zscr[:, 0:8].rearrange("p (h e) -> p h e", e=1)
            if use_act:
                nc.scalar.activation(v3[:, :, 0:64], ps3, AF.Copy,
                                     bias=0.0, scale=vmcol)
                nc.scalar.activation(v3[:, :, 64:65], z3, AF.Identity,
                                     bias=vmcol, scale=1.0)
            else:
                nc.vector.tensor_scalar(v3[:, :, 0:64], ps3, vmcol, None,
                                        op0=OP.mult)
                nc.vector.tensor_scalar(v3[:, :, 64:65], z3, vmcol, None,
                                        op0=OP.add)

        def vproj(vtiles, src_slice, w_all, bias_row, vmask, pp=None,
                  psum_tag="vproj", tok_range=None, use_act=False):
            """Row-major V projection with interleaved ones columns.

            vtiles[t]: [P, H*65]; cols h*65..h*65+63 = V features of head h,
            col h*65+64 = 1.0 (softmax-denominator trick). Rows of masked
            keys are zeroed via the per-partition 0/1 column vmask[:, t]
            (folds the key-padding bias out of the Exp)."""
            for t in tok_range if tok_range is not None else range(len(vtiles)):
                ps = pp.tile([P, D], F32, tag=psum_tag, name="vps")
                for c in range(DC):
                    nc.tensor.matmul(ps[:], lhsT=src_slice(c, t),
                                     rhs=w_all[:, c, :],
                                     start=(c == 0), stop=False)
                nc.tensor.matmul(ps[:], lhsT=ones_row[:], rhs=bias_row[:],
                                 start=False, stop=True)
                vproj_tile(vtiles[t], ps, vmask[:, t: t + 1], use_act)

        def attention(kT, vtiles, qT, use_masks, mergedT, pools, spbufs=3):
            """Per head: all paired score matmuls first, then the AV
            accumulation, then denominator broadcast + merge.
            Key-padding is folded into zeroed V rows, so the Exp bias is 0."""
            sp, avp, rp, sbp = pools

            def emit_scores(h):
                hc, off = h // 2, (h % 2) * DKH
                pts = []
                for pr in range(4):
                    s = sp.tile([P, 2, TOWN], F32, tag="s", bufs=spbufs)
                    for i in range(2):
                        kc = 2 * pr + i
                        nc.tensor.matmul(
                            s[:, i, :], lhsT=kT[hc][off:off + DKH, ts(kc, P)],
                            rhs=qT[hc][off:off + DKH, :], start=True,
                            stop=True)
                    pt = sbp.tile([P, 2, TOWN], MF, tag="pT", bufs=4)
                    nc.scalar.activation(pt[:], s[:], AF.Exp,
                                         bias=0.0, scale=0.125)
                    if use_masks and pr >= 2:
                        # causal triangle on the diagonal 128x128 block;
                        # columns left of it are never read by the AV below
                        for i in range(2):
                            kc = 2 * pr + i
                            q0 = (kc - 4) * P
                            nc.vector.tensor_tensor(
                                pt[:, i, q0:q0 + P], pt[:, i, q0:q0 + P],
                                masks[:, :], op=OP.mult)
                    pts.append(pt)
                return pts

            def emit_avs(h, pts):
                av = avp.tile([DKH + 1, TOWN], F32, tag="av")
                for kc in range(8):
                    q0 = (kc - 4) * P if use_masks and kc >= 4 else 0
                    nc.tensor.matmul(av[:, q0:],
                                     lhsT=vtiles[kc][:, h * 65: h * 65 + 65],
                                     rhs=pts[kc // 2][:, kc % 2, q0:],
                                     start=(kc == 0), stop=(kc == 7),
                                     skip_group_check=True)
                srow = sbp.tile([1, TOWN], MF, tag="srow", bufs=2)
                nc.scalar.copy(srow[:], av[DKH: DKH + 1, :])
                return av, srow

            def emit_rmerge(h, av, srow):
                hc, off = h // 2, (h % 2) * DKH
                R = rp.tile([DKH, TOWN], F32, tag="R")
                nc.tensor.matmul(R[:], lhsT=ones_row[:, 0:DKH],
                                 rhs=srow[:], start=True, stop=True)
                rinv = sbp.tile([DKH, TOWN], F32, tag="rinv", bufs=2)
                nc.vector.reciprocal_approx_fast(rinv[:], R[:])
                nc.vector.tensor_tensor(mergedT[hc][off:off + DKH, :],
                                        av[0:DKH, :], rinv[:], op=OP.mult)

            for h in range(H):
                pts = emit_scores(h)
                av, srow = emit_avs(h, pts)
                emit_rmerge(h, av, srow)

        def layernorm(srcs, i, dsts, pools, filler=None, pad_pool=None,
                      pads=(8, 10)):
            """dsts = LN(srcs) with gain/bias pack i (feature-major chunks).
            `filler` emits independent PE work between the stat matmuls and
            the broadcast matmuls so the PE never dips while ACT/DVE compute
            the row statistics (a PE dip triggers a hardware k=4 window)."""
            statp, bcp, sbp = pools
            ssum = statp.tile([1, TOWN], F32, tag="ssum")
            ssq = statp.tile([1, TOWN], F32, tag="ssq")
            for c in range(DC):
                nc.tensor.matmul(ssum[:], lhsT=ones_col[:], rhs=srcs[c][:],
                                 start=(c == 0), stop=(c == DC - 1))
            for c in range(DC):
                sq = sbp.tile([P, TOWN], MF, tag="sq", bufs=2)
                nc.scalar.square(sq[:], srcs[c][:])
                nc.tensor.matmul(ssq[:], lhsT=ones_col[:], rhs=sq[:],
                                 start=(c == 0), stop=(c == DC - 1))
            mu = sbp.tile([1, TOWN], MF, tag="row", bufs=3, name="mu")
            nc.scalar.mul(mu[:], ssum[:], 1.0 / D)
            musq = sbp.tile([1, TOWN], F32, tag="row", bufs=3, name="musq")
            nc.vector.scalar_tensor_tensor(musq[:], in0=mu[:], scalar=1.0,
                                           in1=mu[:], op0=OP.mult, op1=OP.mult)
            var = sbp.tile([1, TOWN], F32, tag="row", bufs=3, name="var")
            nc.vector.scalar_tensor_tensor(var[:], in0=ssq[:], scalar=1.0 / D,
                                           in1=musq[:], op0=OP.mult,
                                           op1=OP.subtract)
            std = sbp.tile([1, TOWN], MF, tag="row", bufs=3, name="std")
            nc.scalar.activation(std[:], var[:], AF.Sqrt, bias=eps_c[:], scale=1.0)
            def pad(n):
                if pad_pool is None or n == 0:
                    return
                pt = pad_pool.tile([P, TOWN], F32, tag="proj")
                for _ in range(n):
                    nc.tensor.matmul(pt[:, 0:P], lhsT=ones_row[:],
                                     rhs=masks[0:1, 0:P],
                                     start=True, stop=True)
            if filler is not None:
                filler()
            else:
                pad(pads[0])
            mu_b = bcp.tile([P, TOWN], F32, tag="mu_b", bufs=1)
            nc.tensor.matmul(mu_b[:], lhsT=ones_row[:], rhs=mu[:],
                             start=True, stop=True)
            std_b = bcp.tile([P, TOWN], F32, tag="std_b", bufs=1)
            nc.tensor.matmul(std_b[:], lhsT=ones_row[:], rhs=std[:],
                             start=True, stop=True)
            ainv = sbp.tile([P, TOWN], F32, tag="ainv", bufs=1)
            nc.vector.reciprocal_approx_fast(ainv[:], std_b[:])
            pad(pads[1])
            for c in range(DC):
                t1 = sbp.tile([P, TOWN], F32, tag="lnt", bufs=2)
                nc.vector.tensor_tensor(t1[:], srcs[c][:], mu_b[:], op=OP.subtract)
                t2 = sbp.tile([P, TOWN], F32, tag="lnt2", bufs=2)
                nc.vector.tensor_tensor(t2[:], t1[:], ainv[:], op=OP.mult)
                nc.scalar.activation(dsts[c][:], t2[:], AF.Identity,
                                     bias=bc[i][:, c: c + 1],
                                     scale=gc[i][:, c: c + 1])

        # ====== blocks A+B: embed, self-attn (overlapped with enc/ek/ev), =====
        # ====== LN1, cross-attn, LN2                                       =====
        with tc.tile_pool(name="blkB", bufs=1) as bB:
            ekT = [bB.tile([P, T], MF, tag=f"ekT{c}", name=f"ekT{c}") for c in range(DC)]
            evsb = [bB.tile([P, H * 65], MF, tag=f"ev{t}", name=f"ev{t}") for t in range(8)]
            cqT = [bB.tile([P, TOWN], MF, tag=f"cqT{c}", name=f"cqT{c}") for c in range(DC)]
            mergedT2 = [bB.tile([P, TOWN], MF, tag=f"mg2T{c}", name=f"mg2T{c}") for c in range(DC)]

            encp_cm = tc.tile_pool(name="encp", bufs=4)
            encp = encp_cm.__enter__()

            enc_tiles = {}

            def emit_enc_load(th, mp):
                """Encoder half th: load + transpose enc tokens (PE filler)."""
                encTh = []
                for c in range(DC):
                    e = encp.tile([P, TOWN], MF, tag="encTh",
                                  name=f"encTh{th}{c}")
                    encTh.append(e)
                es_all = encp.tile([P, 4, D], F32, tag="es_all",
                                   name=f"es{th}", bufs=2)
                nc.sync.dma_start(
                    es_all[:],
                    a["enc"].rearrange("(g t p) d -> p (g t) d",
                                       g=2, p=P)[:, th * 4:(th + 1) * 4, :])
                for t in range(4):
                    for c in range(DC):
                        tp = mp.tile([P, P], F32, tag="misc", name="tp")
                        nc.tensor.transpose(tp[:], es_all[:, t, ts(c, P)],
                                            ident[:])
                        nc.vector.tensor_copy(encTh[c][:, ts(t, P)], tp[:])
                enc_tiles[th] = encTh

            def emit_enc_proj(th, mp):
                """Encoder half th: eK / eV projections."""
                encTh = enc_tiles.pop(th)
                for m in range(DC):
                    ps = mp.tile([P, TOWN], F32, tag="misc", name="ekp")
                    for c in range(DC):
                        nc.tensor.matmul(
                            ps[:], lhsT=ewk_all[:, c, ts(m, P)],
                            rhs=encTh[c][:, :],
                            start=(c == 0), stop=(c == DC - 1))
                    nc.vector.tensor_scalar(
                        ekT[m][:, th * 512:(th + 1) * 512], ps[:],
                        ebk_c[:, m: m + 1], None, op0=OP.add)
                vproj(evsb, lambda c, t: encTh[c][:, ts(t - th * 4, P)],
                      ewv_all, ebv_r, vmC, pp=mp, psum_tag="misc",
                      tok_range=range(th * 4, th * 4 + 4))

            with tc.tile_pool(name="blkA", bufs=1) as bA, \
                 tc.tile_pool(name="rotA", bufs=3) as rA:
                x0p = [bA.tile([P, TOWN], MF, tag=f"x0p{c}", name=f"x0p{c}") for c in range(DC)]
                x0o = [bA.tile([P, TOWN], MF, tag=f"x0o{c}", name=f"x0o{c}") for c in range(DC)]
                kT = [bA.tile([P, T], MF, tag=f"kT{c}", name=f"kT{c}") for c in range(DC)]
                vsb = [bA.tile([P, H * 65], MF, tag=f"v{t}", name=f"v{t}") for t in range(8)]
                qT = [bA.tile([P, TOWN], MF, tag=f"qT{c}", name=f"qT{c}") for c in range(DC)]
                mergedT = [bA.tile([P, TOWN], MF, tag=f"mgT{c}", name=f"mgT{c}") for c in range(DC)]

                def x0slice(c, t):
                    return (x0p[c][:, ts(t, P)] if t < 4
                            else x0o[c][:, ts(t - 4, P)])

                # --- embedding gather + transpose + scale + pos encoding ---
                with tc.tile_pool(name="psA0", bufs=3, space="PSUM") as pp0:
                    for t in range(8):
                        xg = rA.tile([P, D], F32, tag="xg", bufs=8)
                        nc.gpsimd.indirect_dma_start(
                            out=xg[:], out_offset=None, in_=a["emb"][:, :],
                            in_offset=bass.IndirectOffsetOnAxis(
                                ap=idx_sb[:, t: t + 1], axis=0))
                        for c in range(DC):
                            tp = pp0.tile([P, P], F32, tag="tp")
                            nc.tensor.transpose(tp[:], xg[:, ts(c, P)], ident[:])
                            nc.vector.scalar_tensor_tensor(
                                x0slice(c, t), in0=tp[:], scalar=SQRT_D,
                                in1=peT_sb[:, c, ts(t, P)],
                                op0=OP.mult, op1=OP.add)

                # --- K, V, Q projections ---
                with tc.tile_pool(name="psA1", bufs=3, space="PSUM") as pp1:
                    proj_fm(kT, [x0p, x0o], wk_all, bk_c, pp=pp1)
                    vproj(vsb, x0slice, wv_all, bv_r, vmS, pp=pp1,
                          use_act=True)
                    proj_fm(qT, [x0o], wq_all, bq_c, pp=pp1)

                # --- self-attention overlapped with enc transpose + ek/ev ---
                with tc.tile_pool(name="psS", bufs=2, space="PSUM") as sp, \
                     tc.tile_pool(name="psAV", bufs=2, space="PSUM") as avp, \
                     tc.tile_pool(name="psR", bufs=1, space="PSUM") as rp, \
                     tc.tile_pool(name="psMisc", bufs=1, space="PSUM") as mp, \
                     tc.tile_pool(name="sbA", bufs=3) as sbp:
                    attention(kT, vsb, qT, True, mergedT,
                              (sp, avp, rp, sbp), spbufs=2)
                    # gap filler: encoder half 0 transpose + eK/eV projections
                    emit_enc_load(0, mp)
                    emit_enc_proj(0, mp)

                # --- Wo1 + residual + LN1 -> x1T (enc half 1 fills LN1) ---
                with tc.tile_pool(name="psA2", bufs=2, space="PSUM") as pp2, \
                     tc.tile_pool(name="psStat", bufs=1, space="PSUM") as statp, \
                     tc.tile_pool(name="psBC", bufs=2, space="PSUM") as bcp, \
                     tc.tile_pool(name="psMisc1", bufs=2, space="PSUM") as mp1, \
                     tc.tile_pool(name="sbLN", bufs=3) as sbp:
                    ln_in = []
                    for m in range(DC):
                        ps = pp2.tile([P, TOWN], F32, tag="proj")
                        for c in range(DC):
                            nc.tensor.matmul(ps[:], lhsT=wo1_all[:, c, ts(m, P)],
                                             rhs=mergedT[c][:],
                                             start=(c == 0), stop=(c == DC - 1))
                        li = sbp.tile([P, TOWN], MF, tag="li", bufs=4,
                                      name=f"li{m}")
                        nc.vector.scalar_tensor_tensor(
                            li[:], in0=ps[:], scalar=bo1_c[:, m: m + 1],
                            in1=x0o[m][:], op0=OP.add, op1=OP.add)
                        ln_in.append(li)
                    layernorm(ln_in, 1, x1T, (statp, bcp, sbp),
                              filler=lambda: emit_enc_load(1, mp1))
                    emit_enc_proj(1, mp1)

            # --- cross-attention ---
            with tc.tile_pool(name="psB1", bufs=3, space="PSUM") as pp1:
                proj_fm(cqT, [x1T], cwq_all, cbq_c, pp=pp1)
            encp_cm.__exit__(None, None, None)

            with tc.tile_pool(name="psS", bufs=2, space="PSUM") as sp, \
                 tc.tile_pool(name="psAV", bufs=2, space="PSUM") as avp, \
                 tc.tile_pool(name="psR", bufs=2, space="PSUM") as rp, \
                 tc.tile_pool(name="sbB", bufs=3) as sbp:
                attention(ekT, evsb, cqT, False, mergedT2,
                          (sp, avp, rp, sbp), spbufs=2)

            with tc.tile_pool(name="psB2", bufs=2, space="PSUM") as pp2, \
                 tc.tile_pool(name="psStat", bufs=1, space="PSUM") as statp, \
                 tc.tile_pool(name="psBC", bufs=2, space="PSUM") as bcp, \
                 tc.tile_pool(name="sbLN", bufs=3) as sbp:
                ln_in = []
                for m in range(DC):
                    ps = pp2.tile([P, TOWN], F32, tag="proj")
                    for c in range(DC):
                        nc.tensor.matmul(ps[:], lhsT=wo2_all[:, c, ts(m, P)],
                                         rhs=mergedT2[c][:],
                                         start=(c == 0), stop=(c == DC - 1))
                    li = sbp.tile([P, TOWN], MF, tag=f"li{m}", name=f"li{m}",
                                  bufs=1)
                    nc.vector.scalar_tensor_tensor(
                        li[:], in0=ps[:], scalar=bo2_c[:, m: m + 1],
                        in1=x1T[m][:], op0=OP.add, op1=OP.add)
                    ln_in.append(li)
                layernorm(ln_in, 2, x2T, (statp, bcp, sbp), pad_pool=pp2,
                          pads=(24, 28))

        # ================= blocks C+D =================
        with tc.tile_pool(name="late", bufs=1) as latep, \
             tc.tile_pool(name="wD", bufs=3) as wdp:
          x3T = [latep.tile([P, TOWN], MF, tag=f"x3T{c}", name=f"x3T{c}")
                 for c in range(DC)]

          wout_tiles = {}

          def wout_tile(vg):
              if vg in wout_tiles or vg >= NVG:
                  return wout_tiles.get(vg)
              w_all = wdp.tile([P, DC, VG * VCH], BF16, tag="wo")
              nc.sync.dma_start(
                  w_all[:],
                  a["WoutT"].rearrange("(c p) v -> p c v", p=P)
                  [:, :, vg * VG * VCH:(vg + 1) * VG * VCH])
              wout_tiles[vg] = w_all
              return w_all

          # ----- block C: FFN, LN3 -----
          with tc.tile_pool(name="hC", bufs=1) as hp, \
               tc.tile_pool(name="psC", bufs=4, space="PSUM") as pp, \
               tc.tile_pool(name="psStat", bufs=1, space="PSUM") as statp, \
               tc.tile_pool(name="psBC", bufs=2, space="PSUM") as bcp, \
               tc.tile_pool(name="sbC", bufs=3) as sbp:
              # prefetch the first two vocab weight groups during the FFN
              wout_tile(0)
              wout_tile(1)
              hT = [hp.tile([P, TOWN], MF, tag=f"hT{m}", name=f"hT{m}") for m in range(FFN // P)]
              for w in range(2):
                  wps = [pp.tile([P, 512], F32, tag="proj",
                                 name=f"w1ps{w}{j}") for j in range(4)]
                  for c in range(DC):
                      for j in range(4):
                          m = 4 * w + j
                          nc.tensor.matmul(wps[j][:],
                                           lhsT=w1_all[:, c, ts(m, P)],
                                           rhs=x2T[c][:],
                                           start=(c == 0), stop=(c == DC - 1))
                  for j in range(4):
                      m = 4 * w + j
                      nc.scalar.activation(hT[m][:], wps[j][:], AF.Relu,
                                           bias=b1_c[:, m: m + 1], scale=1.0)
              ln_in = []
              for m in range(DC):
                  ps = pp.tile([P, TOWN], F32, tag="proj")
                  for c in range(FFN // P):
                      nc.tensor.matmul(ps[:], lhsT=w2_all[:, c, ts(m, P)],
                                       rhs=hT[c][:],
                                       start=(c == 0), stop=(c == FFN // P - 1))
                  li = sbp.tile([P, TOWN], MF, tag=f"li{m}", name=f"li{m}", bufs=1)
                  nc.vector.scalar_tensor_tensor(
                      li[:], in0=ps[:], scalar=b2_c[:, m: m + 1], in1=x2T[m][:],
                      op0=OP.add, op1=OP.add)
                  ln_in.append(li)
              layernorm(ln_in, 3, x3T, (statp, bcp, sbp), pad_pool=pp,
                        pads=(24, 36))

          # ================= block D: vocab projection =================
          with tc.tile_pool(name="stD", bufs=6) as stp, \
               tc.tile_pool(name="psD", bufs=2, space="PSUM") as pp:
              outr = a["out"].rearrange("(t p) v -> p t v", p=P)
              for vg in range(NVG):
                  w_all = wout_tile(vg)
                  wout_tile(vg + 1)                  # rolling prefetch
                  del wout_tiles[vg]
                  for t in range(TOWN // P):
                      ps = pp.tile([P, VG, 512], F32, tag="vps")
                      for j in range(VG):
                          for c in range(DC):
                              nc.tensor.matmul(
                                  ps[:, j, 0:VCH],
                                  lhsT=x3T[c][:, ts(t, P)],
                                  rhs=w_all[:, c, ts(j, VCH)],
                                  start=(c == 0), stop=(c == DC - 1))
                      stage = stp.tile([P, VG * VCH], OUT_DT, tag="stage")
                      st3 = stage[:].rearrange("p (j e) -> p j e", e=VCH)
                      if t % 2 == 0:
                          nc.scalar.copy(st3, ps[:, :, 0:VCH])
                      else:
                          nc.vector.tensor_copy(st3, ps[:, :, 0:VCH])
                      nc.sync.dma_start(
                          outr[:, t, vg * VG * VCH:(vg + 1) * VG * VCH],
                          stage[:])


# --------------------------------------------------------------------------
# host-side input preparation
# --------------------------------------------------------------------------

def _pos_encoding_np(t, d):
    pos = np.arange(t, dtype=np.float32)[:, None]
    freqs = 1.0 / (10000.0 ** (np.arange(0, d, 2, dtype=np.float32) / d))
    pe = np.zeros((t, d), np.float32)
    pe[:, 0::2] = np.sin(pos * freqs)
    pe[:, 1::2] = np.cos(pos * freqs)
    return pe


def _col_pack(b):
    """[n] -> [P, n//P] with element (p, c) = b[c*P + p]."""
    b = np.asarray(b, np.float32)
    return np.ascontiguousarray(b.reshape(-1, P).T)


def prep_in_maps(inputs):
    import ml_dtypes
    bf16 = ml_dtypes.bfloat16
    gi = lambda n: np.asarray(inputs[n])
    tokens = gi("tokens").astype(np.int32)                      # [4, 1024]
    enc_all = np.ascontiguousarray(gi("enc_embeddings").astype(np.float32))
    enc_pad = gi("enc_pad_mask").astype(bool)
    emb = np.ascontiguousarray(gi("emb").astype(np.float32))

    shared = {"emb": emb}
    for nm in ("Wq", "Wk", "Wv", "Wo1", "cWq", "eWk", "eWv", "Wo2", "W1", "W2",
               "Wout"):
        shared[nm + "T"] = np.ascontiguousarray(
            gi(nm).astype(np.float32).T).astype(bf16)
    shared["bv_r"] = gi("bv").astype(np.float32).reshape(1, D).astype(bf16)
    shared["ebv_r"] = gi("ebv").astype(np.float32).reshape(1, D).astype(bf16)

    # packed fp32 constant block (per-core biasS/biasC patched below)
    cblk = np.zeros((P, NCONST), np.float32)
    def setc(nm, arr):
        o, n = _CB[nm]
        cblk[:, o:o + n] = arr
    for nm, src in (("bq", "bq"), ("bk", "bk"), ("bo1", "bo1"), ("cbq", "cbq"),
                    ("ebk", "ebk"), ("bo2", "bo2"), ("b2", "b2"), ("b1", "b1")):
        setc(nm, _col_pack(gi(src)))
    for i, (g, b) in ((1, ("g1", "be1")), (2, ("g3", "be3")), (3, ("g2", "be2"))):
        setc(f"gc{i}", _col_pack(gi(g)))
        setc(f"bc{i}", _col_pack(gi(b)))

    # causal 0/1 lower-triangle for the diagonal 128-blocks (key <= query)
    kk = np.arange(P)[:, None]
    qq = np.arange(P)[None, :]
    shared["masks"] = np.where(kk <= qq, 1.0, 0.0).astype(bf16)

    pe = _pos_encoding_np(T, D)

    in_maps = []
    for core in range(8):
        b, hf = core // 2, core % 2
        own = tokens[b, hf * 512:(hf + 1) * 512]
        idx_full = np.concatenate([tokens[b, :512], own])        # [1024]
        pe_slots = np.concatenate([pe[:512], pe[hf * 512:(hf + 1) * 512]], axis=0)
        peT = np.ascontiguousarray(
            pe_slots.T.reshape(DC, P, T, order="C"))
        # pe_slots.T is [D, T]; reshape to [DC, P, T] splits D into chunks
        vmS = np.where(idx_full == PAD_ID, 0.0, 1.0).astype(np.float32)
        if hf == 0:
            vmS[:512] = 0.0                                      # no prefix half
        vmC = np.where(enc_pad[b], 0.0, 1.0).astype(np.float32)
        m = dict(shared)
        m["idx"] = np.ascontiguousarray(idx_full.reshape(T, 1))
        m["peT"] = peT.astype(bf16)
        m["enc"] = np.ascontiguousarray(enc_all[b])
        cb = cblk.copy()
        cb[:, _CB["vmS"][0]:_CB["vmS"][0] + 8] = \
            np.ascontiguousarray(vmS.reshape(8, P).T)
        cb[:, _CB["vmC"][0]:_CB["vmC"][0] + 8] = \
            np.ascontiguousarray(vmC.reshape(8, P).T)
        m["consts"] = cb
        in_maps.append(m)
    return in_maps


def assemble(results, inputs):
    full = np.empty((4, 1024, V), np.float32)
    for core in range(8):
        b, hf = core // 2, core % 2
        full[b, hf * 512:(hf + 1) * 512] = np.asarray(
            results[core]["out"]).astype(np.float32)
    bout = np.asarray(inputs["bout"], np.float32)
    if np.any(bout):
        full += bout[None, None, :]
    return full


# --------------------------------------------------------------------------
# public entry point
# --------------------------------------------------------------------------

def kernel(**inputs):
    from concourse.bass_utils import run_bass_kernel_spmd
    nc = build_module()
    in_maps = prep_in_maps(inputs)
    res = run_bass_kernel_spmd(nc, in_maps, core_ids=list(range(8)))
    return assemble(res.results, inputs)


if __name__ == "__main__":
    nc = build_module()
    print("built ok")
